# revision 19
# baseline (speedup 1.0000x reference)
"""Trainium2 Bass kernel for masked cosine attention (nn_Native_Attention_msa).

Shape: B=2, N=2048, C=1024, H=16 heads, hd=64.
Sharding: 8 cores = 2 batches x 4 head-groups (4 heads per core).
Each core computes qkv for its heads, cosine attention, and a partial
projection (its 256 columns of proj_w); host sums the 4 partials per batch.

Tokens are sorted by cls_score on the host (attention is permutation
invariant over keys; queries permuted identically and undone on output),
so the mask  mask[i,j] = (s_j > s_i - 0.1)  becomes a monotone staircase:
per key j, masked queries are a prefix  i < b_j  with b_j computed on the
host (searchsorted). Fully-masked (i,j) tiles contribute exp(0)=1 and are
folded in via column sums of V; fully-clear tiles need no mask at all;
boundary tiles apply the mask inside a custom DVE op using the Idx stream
counter against b' = b - 512*it (no mask tensor, no mask build).

exp() is computed two ways and load-balanced across engines:
 - ACT (scalar engine): hardware Exp on [128, 2*512] (clear tiles)
 - DVE (vector engine): custom fused ops; |z| <= 0.125 (cosine attention
   * scale 1/8 * s in [0,1)), so a tuned quadratic/cubic Taylor poly has
   max rel err < 1e-4:
     EXP_PAGED_ANT: (Idx < b')*(z*(c1 + z*c0)) + 1   [masked boundary]
     EXP_POLY3_ANT: 1 + z*(1 + z*(1/2 + z/6))        [clear tiles]

All device compute in fp16 with fp32 PSUM accumulation.
"""

import sys
import numpy as np

sys.path.insert(0, "/opt/trn_rl_repo")

N = 2048
C = 1024
H = 16
HD = 64
B = 2
NCORES = 8
HPC = 4          # heads per core
NTI = 4          # i tiles of 512
TI = 512
NTJ = 16         # j tiles of 128
TJ = 128
KC = 8           # c tiles of 128 for qkv
SCALE = HD ** -0.5

# exp-poly coefficients (|z| <= 0.125)
EXPQ_C0 = 0.5
EXPQ_C1 = 1.001953125     # minimax-tuned linear coeff, max rel err 8.8e-5
EXP3_C0 = 0.5
EXP3_C1 = 1.0 / 6.0       # cubic Taylor, max rel err 1.1e-5

_CACHE = {}
_DVE_OPS = {}


def _register_dve_ops():
    """Register the two custom DVE exp ops (idempotent)."""
    if _DVE_OPS:
        return _DVE_OPS
    import concourse.dve_ops as dops
    from concourse.dve_spec import (
        Spec, Src0, C0, C1, C2, C3, One, Idx,
        lower as dve_lower, _has_src1, _spill_c3_to_src1,
    )
    from concourse.dve_uop import DveOpSpec

    def ref_paged(in0, in1, c0, c1, c2):
        z = in0.astype(np.float32)
        q = (z * (c1 + z * c0)).astype(np.float32)
        idx = np.arange(in0.shape[-1], dtype=np.float32)
        m = (idx[None, :] < np.asarray(in1, np.float32).reshape(-1, 1))
        return (m.astype(np.float32) * q + c2).astype(np.float32)

    def ref_poly3(in0, in1, c0, c1, c2):
        z = in0.astype(np.float32)
        return (c2 + z * (1.0 + z * (c0 + z * c1))).astype(np.float32)

    body_paged = _spill_c3_to_src1(
        (Idx < C3) * (Src0 * (C1 + Src0 * C0)) + C2)
    body_poly3 = C2 + Src0 * (One + Src0 * (C0 + Src0 * C1))

    for name, body, ref in (
        ("EXP_PAGED_ANT", body_paged, ref_paged),
        ("EXP_POLY3_ANT", body_poly3, ref_poly3),
    ):
        if name in dops._SUB_OPCODE_FOR_NAME:
            _DVE_OPS[name] = next(o for o in dops.OPS if o.name == name)
            continue
        spec = Spec(body=body, reference=ref)
        row = dops._CUSTOM_DVE_ROW_BASE + len(dops.OPS)
        uops = dve_lower(spec, ver="v3")
        s = DveOpSpec(name=name, opcode=row, uops=uops,
                      rd1_en=_has_src1(spec))
        op = dops.DveOp(name=name, spec=spec, subdim=False,
                        uops_sha={"v3": s.sha("v3")})
        dops.OPS.append(op)
        dops._SUB_OPCODE_FOR_NAME[name] = row
        dops.CUSTOM_DVE_SPECS[name] = spec
        _DVE_OPS[name] = op
    return _DVE_OPS


# measured per-op engine costs (ns) for the ACT/DVE load balancer
_ACT_EXP2 = 1113.0      # Exp on [128, 2*512] from PSUM
_ACT_EXPH = 678.0       # Exp on [128, 512] from PSUM
_DVE_PAGED2 = 1300.0    # 2x EXP_PAGED on [128,512] from PSUM
_DVE_POLY2 = 1224.0     # EXP_POLY3 on [128, 2*512] from PSUM
_DVE_POLYH = 690.0      # EXP_POLY3 on [128, 512] from PSUM
_ACT_CP512 = 686.0      # scalar Copy [128,512]
_DVE_TS512 = 680.0      # vector op [128,512] psum-src
_ACT_CP256 = 450.0
_DVE_CP256 = 392.0


def _build(use_mask: bool, tilecls=None):
    import concourse.bass as bass
    import concourse.bacc as bacc
    import concourse.mybir as mybir
    import concourse.tile as tile
    from contextlib import ExitStack

    ops = _register_dve_ops()
    EXP_PAGED = ops["EXP_PAGED_ANT"]
    EXP_POLY3 = ops["EXP_POLY3_ANT"]

    if tilecls is None:
        tilecls = ((0, 16),) * NTI if use_mask else ((0, 0),) * NTI
    jz_max = max(jz for jz, _ in tilecls)
    any_zero = jz_max > 0

    dt = mybir.dt
    f32 = dt.float32
    f16 = dt.float16
    Alu = mybir.AluOpType
    Act = mybir.ActivationFunctionType

    nc = bacc.Bacc("TRN2", target_bir_lowering=False, debug=False,
                   num_devices=NCORES)

    xt_d = nc.dram_tensor("xt", [C, N], f16, kind="ExternalInput").ap()
    wq_d = nc.dram_tensor("wqkvT", [C, 768], f16, kind="ExternalInput").ap()
    pw_d = nc.dram_tensor("pwT", [256, C], f16, kind="ExternalInput").ap()
    cls_d = nc.dram_tensor("cls", [N], f32, kind="ExternalInput").ap()
    bcol_d = nc.dram_tensor("bcol", [128, NTI * NTJ], f32,
                            kind="ExternalInput").ap()
    yt_d = nc.dram_tensor("yT", [C, N], f16, kind="ExternalOutput").ap()

    # engine load balancer (build-time estimates)
    eng_ns = {"act": 0.0, "dve": 0.0}

    def pick(act_cost, dve_cost):
        if eng_ns["act"] + act_cost <= eng_ns["dve"] + dve_cost:
            eng_ns["act"] += act_cost
            return "act"
        eng_ns["dve"] += dve_cost
        return "dve"

    with tile.TileContext(nc) as tc, ExitStack() as ctx:
        pool = ctx.enter_context(tc.tile_pool(name="persist", bufs=1))
        qhat = pool.tile([128, 2, N], f16)
        khat = pool.tile([128, 2, N], f16)
        vt = pool.tile([128, NTJ, HPC, HD], f16)
        outT = pool.tile([128, 2, N], f16)
        pw_sb = pool.tile([128, 2, C], f16)
        onesT = pool.tile([128, 64], f16)
        ones128 = pool.tile([128, 128], f16)
        onesF = pool.tile([1, 128], f32)
        ss_full = pool.tile([128, N], f32)
        bcol_sb = pool.tile([128, NTI, NTJ], f32)
        vsum_all = pool.tile([128, 2, max(jz_max, 1)], f32)

        # --- phase A pools (released before phase B) ---
        a_ctx = ExitStack()
        smpool = a_ctx.enter_context(tc.tile_pool(name="smp", bufs=1))
        cls_sb = smpool.tile([1, N], f32)
        xpool = a_ctx.enter_context(tc.tile_pool(name="xp", bufs=1))
        xt_sb = xpool.tile([128, KC, N], f16)
        wpool = a_ctx.enter_context(tc.tile_pool(name="wp", bufs=1))
        wq_sb = wpool.tile([128, KC, 768], f16)
        sqpool = a_ctx.enter_context(tc.tile_pool(name="sqp", bufs=3))

        # input DMAs. xt arrives in n-major quarter-chunks so the qk loop
        # (n outer) is never gated on the last k-chunk; wq rides the gpsimd
        # ring in parallel with xt's even-k chunks on the sync ring.
        nc.sync.dma_start(out=cls_sb, in_=cls_d.rearrange("(a n) -> a n", a=1))
        nc.sync.dma_start(
            out=bcol_sb,
            in_=bcol_d.rearrange("p (i j) -> p i j", i=NTI))
        for k in range(KC):
            nc.gpsimd.dma_start(out=wq_sb[:, k, :], in_=wq_d[k * 128:(k + 1) * 128, :])
        for nq in range(NTI):
            nsl = slice(nq * TI, (nq + 1) * TI)
            for k in range(KC):
                q = (nc.sync if k % 2 == 0 else nc.gpsimd)
                q.dma_start(out=xt_sb[:, k, nsl],
                            in_=xt_d[k * 128:(k + 1) * 128, nsl])
        for k in range(2):
            nc.gpsimd.dma_start(out=pw_sb[:, k, :], in_=pw_d[k * 128:(k + 1) * 128, :])

        # constants
        nc.vector.memset(onesT, 1.0)
        nc.vector.memset(onesF, 1.0)
        nc.vector.memset(ones128, 0.0)
        nc.vector.memset(ones128[0:64, 0:64], 1.0)
        nc.vector.memset(ones128[64:128, 64:128], 1.0)

        # PE warm-up burst while input DMAs land, so HAM reaches K=8/8
        # before real work starts.
        with tc.tile_pool(name="wup", bufs=2, space="PSUM") as wu_pool, \
             tc.tile_pool(name="wsb", bufs=1) as ws_pool:
            wsrc = ws_pool.tile([128, TI], f16)
            nc.vector.memset(wsrc, 1.0)
            for _ in range(8):
                wu = wu_pool.tile([128, TI], f32)
                for r in range(2):
                    nc.tensor.matmul(wu, lhsT=ones128, rhs=wsrc,
                                     start=(r == 0), stop=(r == 1))

        # s broadcast -> ss_full rows = 0.125 * s_j (folded into khat)
        with tc.tile_pool(name="bcps", bufs=2, space="PSUM") as bc_ps_pool:
            for n in range(NTI):
                bc = bc_ps_pool.tile([128, TI], f32)
                nc.tensor.matmul(bc, lhsT=onesF,
                                 rhs=cls_sb[:, n * TI:(n + 1) * TI],
                                 start=True, stop=True)
                nc.vector.tensor_scalar(ss_full[:, n * TI:(n + 1) * TI],
                                        bc, SCALE, None, op0=Alu.mult)

        # --- QKV: q and k (transposed layout [d, n]) ---
        qkv_ps_pool = a_ctx.enter_context(
            tc.tile_pool(name="qkvps", bufs=2, space="PSUM"))
        nrm_ps_pool = a_ctx.enter_context(
            tc.tile_pool(name="nrmps", bufs=2, space="PSUM"))
        rpool = a_ctx.enter_context(tc.tile_pool(name="rp", bufs=3))
        for n in range(NTI):      # n outer: column n needs only xt chunk n
            for m in (0, 2, 1, 3):
                dst = qhat if m < 2 else khat
                g = m % 2
                nsl = slice(n * TI, (n + 1) * TI)
                ps = qkv_ps_pool.tile([128, TI], f32)
                for k in range(KC):
                    nc.tensor.matmul(ps, lhsT=wq_sb[:, k, m * 128:(m + 1) * 128],
                                     rhs=xt_sb[:, k, n * TI:(n + 1) * TI],
                                     start=(k == 0), stop=(k == KC - 1))
                sq = sqpool.tile([128, TI], f16, tag="sq")
                nc.scalar.activation(sq, ps, Act.Square)
                # block-diag ones -> norm^2 replicated: rows 0:64 = even
                # head, rows 64:128 = odd head
                nps = nrm_ps_pool.tile([128, TI], f32)
                nc.tensor.matmul(nps, lhsT=ones128, rhs=sq, start=True,
                                 stop=True)
                rsq = rpool.tile([128, TI], f32, tag="rsq")
                nc.scalar.activation(rsq, nps, Act.Sqrt)
                rb_ = rpool.tile([128, TI], f32, tag="rb")
                nc.vector.reciprocal_approx_fast(rb_, rsq)
                if m >= 2:  # fold 0.125 * s_j into k
                    rbk = rpool.tile([128, TI], f32, tag="rbk")
                    nc.vector.tensor_mul(rbk, rb_, ss_full[:, nsl])
                    rb_ = rbk
                nc.vector.tensor_mul(dst[:, g, nsl], ps, rb_)

        # --- V in natural layout [n, d], scattered per head; vsum matmuls
        # (constant contribution of all-masked tiles) interleaved so they
        # don't serialize at the end ---
        v_ps_pool = a_ctx.enter_context(
            tc.tile_pool(name="vps", bufs=2, space="PSUM"))
        vs_ctx = ExitStack()
        if any_zero:
            vs_pool = vs_ctx.enter_context(
                tc.tile_pool(name="vsps", bufs=1, space="PSUM"))
            vs_ps = vs_pool.tile([128, 2, jz_max], f32)
        for nt in range(NTJ):
            vps = v_ps_pool.tile([128, 256], f32)
            for k in range(KC):
                nc.tensor.matmul(vps, lhsT=xt_sb[:, k, nt * 128:(nt + 1) * 128],
                                 rhs=wq_sb[:, k, 512:768],
                                 start=(k == 0), stop=(k == KC - 1))
            nc.scalar.activation(vt[:, nt, :, :],
                                 vps.rearrange("p (h d) -> p h d", h=HPC),
                                 Act.Copy)
            eng_ns["act"] += _ACT_CP256
            if any_zero and nt < jz_max:
                for g in range(2):
                    nc.tensor.matmul(vs_ps[:, g, nt:nt + 1],
                                     lhsT=vt[:, nt, 2 * g:2 * g + 2, :],
                                     rhs=onesT[:, 0:1],
                                     start=True, stop=True,
                                     skip_group_check=True)
        if any_zero:
            nc.vector.tensor_copy(vsum_all[:, :, 0:jz_max], vs_ps)
            vs_ctx.close()

        a_ctx.close()

        # --- phase B: scores, exp, A@V, denominators, normalize, project.
        # Normalize/projection of group gi-1 is deferred INTO group gi's
        # jt loop so the in-order ACT/DVE queues never stall the exp
        # pipeline at group boundaries. ---
        b_ctx = ExitStack()
        apool = b_ctx.enter_context(tc.tile_pool(name="ap", bufs=6))
        dnpool = b_ctx.enter_context(tc.tile_pool(name="dnp", bufs=6))
        p_ps_pool = b_ctx.enter_context(
            tc.tile_pool(name="pps", bufs=1, space="PSUM"))
        ypool = b_ctx.enter_context(tc.tile_pool(name="ysb", bufs=4))
        c1 = ExitStack()
        z_ps_pool = c1.enter_context(
            tc.tile_pool(name="zps", bufs=4, space="PSUM"))
        av_ps_pool = c1.enter_context(
            tc.tile_pool(name="avps", bufs=2, space="PSUM"))
        d_ps_pool = c1.enter_context(
            tc.tile_pool(name="dps", bufs=1, space="PSUM"))

        st = {}  # per-group deferred state

        def emit_dnb(gi):
            """Free group gi's dn psum bank: copy + bias to SBUF (ACT)."""
            s = st[gi]
            dnb = dnpool.tile([128, TI], f32, tag="dnb")
            nc.scalar.activation(dnb, s["dn"], Act.Copy,
                                 bias=float(128 * s["jz"]))
            eng_ns["act"] += _ACT_CP512
            s["dnb"] = dnb

        def emit_norm(gi):
            """Deferred normalize of group gi (deps met long ago)."""
            s = st[gi]
            rn = dnpool.tile([128, TI], f32, tag="rn")
            nc.vector.reciprocal_approx_fast(rn, s["dnb"])
            if s["jz"] > 0:
                cst = dnpool.tile([128, 1], f32, tag="cst")
                nc.vector.tensor_reduce(cst, vsum_all[:, s["g"], 0:s["jz"]],
                                        axis=mybir.AxisListType.X,
                                        op=Alu.add)
                nc.vector.scalar_tensor_tensor(
                    out=outT[:, s["g"], s["isl"]], in0=s["av"], scalar=cst,
                    in1=rn, op0=Alu.add, op1=Alu.mult)
                eng_ns["dve"] += 2 * _DVE_TS512 + 140.0
            else:
                nc.vector.tensor_mul(outT[:, s["g"], s["isl"]], s["av"], rn)
                eng_ns["dve"] += 2 * _DVE_TS512

        def emit_proj(it_p, ets, pool=None):
            """Projection slice for i-column it_p (outT rows finalized)."""
            pool = pool or p_ps_pool
            isl_ = slice(it_p * TI, (it_p + 1) * TI)
            for et in ets:
                pps = pool.tile([128, TI], f32)
                for k2 in range(2):
                    nc.tensor.matmul(pps,
                                     lhsT=pw_sb[:, k2, et * 128:(et + 1) * 128],
                                     rhs=outT[:, k2, isl_],
                                     start=(k2 == 0), stop=(k2 == 1))
                ysb = ypool.tile([128, TI], f16, tag="y")
                if pick(_ACT_CP512, _DVE_TS512) == "act":
                    nc.scalar.activation(ysb, pps, Act.Copy)
                else:
                    nc.vector.tensor_copy(ysb, pps)
                nc.sync.dma_start(
                    out=yt_d[et * 128:(et + 1) * 128, isl_], in_=ysb)

        def jt_order(jz, jm):
            """Interleave mixed (DVE-bound) and clear (mostly ACT) tiles so
            neither exp engine sees a long same-engine run."""
            mixed = list(range(jz, jm))
            clear = list(range(jm, NTJ))
            order, im, ic = [], 0, 0
            while im < len(mixed) or ic < len(clear):
                if im < len(mixed) and (
                        ic >= len(clear)
                        or im * len(clear) <= ic * len(mixed)):
                    order.append(mixed[im]); im += 1
                else:
                    order.append(clear[ic]); ic += 1
            return order

        # flat triple list (software pipeline: av/dn MMs trail z/exp by 2
        # triples so the in-order PE queue never blocks on an exp)
        triples = []
        for gi in range(2 * NTI):
            it, g = gi // 2, gi % 2
            jz, jm = tilecls[it] if use_mask else (0, 0)
            order = jt_order(jz, jm) if use_mask else list(range(NTJ))
            for pos, jt in enumerate(order):
                triples.append({
                    "gi": gi, "it": it, "g": g, "jt": jt, "jz": jz,
                    "jm": jm, "pos": pos, "first": pos == 0,
                    "last": pos == len(order) - 1,
                    "ntile": len(order)})

        def emit_front(T):
            gi, it, g, jt = T["gi"], T["it"], T["g"], T["jt"]
            isl = slice(it * TI, (it + 1) * TI)
            if T["first"]:
                av = av_ps_pool.tile([128, TI], f32)
                dn_ps = d_ps_pool.tile([128, TI], f32)
                st[gi] = {"av": av, "dn": dn_ps, "jz": T["jz"], "g": g,
                          "isl": isl}
            a2 = apool.tile([128, 2, TI], f16, tag="a")
            # per-hh z psum tiles (1 bank each, pool bufs=4): a bank is
            # recycled as soon as ITS half's exp is done, so ~2.5 triples
            # stay in flight instead of 2
            zh0 = z_ps_pool.tile([128, TI], f32, tag="z")
            zh1 = z_ps_pool.tile([128, TI], f32, tag="z")
            zh = [zh0, zh1]
            T["a2"] = a2
            # scores: row-tiled pair (K=64 halves run concurrently)
            for hh in range(2):
                psl = slice(hh * 64, (hh + 1) * 64)
                nc.tensor.matmul(
                    zh[hh],
                    lhsT=khat[psl, g, jt * TJ:(jt + 1) * TJ],
                    rhs=qhat[psl, g, isl], start=True, stop=True)
            # exp (per half): boundary tiles via DVE paged op; clear tiles
            # load-balanced between ACT Exp and DVE poly
            if use_mask and jt < T["jm"]:
                for hh in range(2):
                    nc.vector._custom_dve(
                        EXP_PAGED, out=a2[:, hh, :],
                        in0=zh[hh],
                        in1=bcol_sb[:, it, jt:jt + 1],
                        s0=EXPQ_C0, s1=EXPQ_C1, imm2=1.0)
                eng_ns["dve"] += _DVE_PAGED2
            else:
                if pick(_ACT_EXPH * 2, _DVE_POLYH * 2) == "act":
                    for hh in range(2):
                        nc.scalar.activation(a2[:, hh, :], zh[hh], Act.Exp)
                else:
                    for hh in range(2):
                        nc.vector._custom_dve(
                            EXP_POLY3, out=a2[:, hh, :], in0=zh[hh],
                            s0=EXP3_C0, s1=EXP3_C1, imm2=1.0)

        def emit_back(T):
            gi, g, jt = T["gi"], T["g"], T["jt"]
            s = st[gi]
            a2 = T["a2"]
            first, last = T["first"], T["last"]
            # A@V pair (col groups 0:1 and 2:3 -> co-execute)
            for hh in range(2):
                nc.tensor.matmul(s["av"][hh * 64:(hh + 1) * 64, :],
                                 lhsT=vt[:, jt, 2 * g + hh, :],
                                 rhs=a2[:, hh, :],
                                 start=first, stop=last,
                                 skip_group_check=True)
            # denominator pair, replicated over each head's rows
            for hh in range(2):
                nc.tensor.matmul(s["dn"][hh * 64:(hh + 1) * 64, :],
                                 lhsT=onesT, rhs=a2[:, hh, :],
                                 start=first, stop=last,
                                 skip_group_check=True)
            if last:
                emit_dnb(gi)

        LAG = 2
        for n, T in enumerate(triples):
            emit_front(T)
            if n >= LAG:
                emit_back(triples[n - LAG])
            # deferred normalize / projection of earlier groups, placed
            # where their deps are long met (queue positions, not barriers)
            gi, pos = T["gi"], T["pos"]
            if gi > 0 and pos == min(4, T["ntile"] - 2):
                emit_norm(gi - 1)
            if T["g"] == 0 and T["it"] >= 1:
                if pos == min(5, T["ntile"] - 1):
                    emit_proj(T["it"] - 1, range(0, 4))
            if T["g"] == 1 and T["it"] >= 1:
                if pos == 1:
                    emit_proj(T["it"] - 1, range(4, 8))
        for T in triples[-LAG:]:
            emit_back(T)
        emit_norm(2 * NTI - 1)
        # free z/av/dn psum banks, then run the last projection on a
        # triple-buffered pool so its matmul pairs and copies pipeline
        c1.close()
        with tc.tile_pool(name="pps2", bufs=3, space="PSUM") as p2:
            emit_proj(NTI - 1, range(0, 8), pool=p2)
        b_ctx.close()

    nc.compile()
    return nc


def _get_nc(use_mask: bool, tilecls=None):
    key = (bool(use_mask), tilecls)
    if key not in _CACHE:
        _CACHE[key] = _build(*key)
    return _CACHE[key]


def _mask_bounds(sp):
    """b[j] = #queries i (sorted order) with mask[i,j]=1, i.e.
    #{i: f32(s_i - 0.1) < s_j}. Monotone staircase since sp ascending."""
    m1 = (sp - np.float32(0.1)).astype(np.float32)
    return np.searchsorted(m1, sp, side="left").astype(np.int64)


def _classify(b):
    """Per i-tile: (jz, jm) = count of fully-masked j-tile prefix, first
    fully-clear j-tile."""
    bt = b.reshape(NTJ, TJ)
    btmax = bt.max(axis=1)
    btmin = bt.min(axis=1)
    out = []
    for it in range(NTI):
        lo, hi = it * TI, (it + 1) * TI
        jz = int(np.sum(btmax <= lo))
        jm = NTJ - int(np.sum(btmin >= hi))
        out.append((jz, max(jm, jz)))
    return tuple(out)


def _prep_in_maps(x_cls, cls_score, qkv_w, proj_w, perm=None, bcol=None):
    in_maps = []
    cls32 = np.ascontiguousarray(cls_score, dtype=np.float32)
    if perm is not None:
        cls32 = np.ascontiguousarray(cls32[perm])
    if bcol is None:
        bcol = np.full((128, NTI * NTJ), 4096.0, dtype=np.float32)
    for c in range(NCORES):
        b, g4 = c // 4, c % 4
        r0 = g4 * 256
        w_cols = np.concatenate([
            qkv_w[r0:r0 + 256],
            qkv_w[C + r0:C + r0 + 256],
            qkv_w[2 * C + r0:2 * C + r0 + 256],
        ], axis=0)  # [768, 1024]
        xb = x_cls[b] if perm is None else x_cls[b][perm]
        in_maps.append({
            "xt": np.ascontiguousarray(xb.T, dtype=np.float16),
            "wqkvT": np.ascontiguousarray(w_cols.T, dtype=np.float16),
            "pwT": np.ascontiguousarray(proj_w[:, r0:r0 + 256].T,
                                        dtype=np.float16),
            "cls": cls32,
            "bcol": bcol,
        })
    return in_maps


def kernel(x_cls, cls_score, qkv_w, proj_w, proj_b, use_mask, _res_hook=None):
    from concourse import bass_utils

    um = int(np.asarray(use_mask)) != 0
    cls32 = np.asarray(cls_score, dtype=np.float32)
    if um:
        # Sort tokens by cls_score: attention is permutation-invariant over
        # keys, and we permute queries identically (undone on output). The
        # mask then becomes a monotone staircase.
        perm = np.argsort(cls32, kind="stable")
        bj = _mask_bounds(cls32[perm])
        tilecls = _classify(bj)
        # b' per (partition p, it, jt): b[jt*128+p] - it*512
        bcol = np.empty((128, NTI * NTJ), dtype=np.float32)
        for it in range(NTI):
            for jt in range(NTJ):
                bcol[:, it * NTJ + jt] = (
                    bj[jt * 128:(jt + 1) * 128] - it * TI).astype(np.float32)
    else:
        perm, tilecls, bcol = None, None, None
    nc = _get_nc(um, tilecls=tilecls)
    in_maps = _prep_in_maps(np.asarray(x_cls, dtype=np.float32),
                            cls32, qkv_w, proj_w, perm=perm, bcol=bcol)
    res = bass_utils.run_bass_kernel_spmd(nc, in_maps,
                                          core_ids=list(range(NCORES)))
    if _res_hook is not None:
        _res_hook(res)
    y = np.zeros((B, N, C), dtype=np.float32)
    for c in range(NCORES):
        y[c // 4] += res.results[c]["yT"].T.astype(np.float32)
    if perm is not None:
        inv = np.empty(N, dtype=np.int64)
        inv[perm] = np.arange(N)
        y = y[:, inv, :]
    y += np.asarray(proj_b, dtype=np.float32)[None, None, :]
    return y
